# revision 1
# baseline (speedup 1.0000x reference)
"""DiffAttn (Differential Transformer attention) on 8 trn2 NeuronCores.

Sharding: tensor-parallel over heads. 16 heads / 8 cores = 2 heads per core.
Wq/Wk/Wv column-sharded (256 cols/core), Wo + x replicated. The reference's
"reshape without transposing heads back" maps output row r = h*128 + (t//16)
entirely to head h, so each core produces rows [256*m, 256*(m+1)) of the
final (2048, 2048) output with NO collectives.

v2 design notes (perf):
- Phase A (xT via PE transpose + Q/K/V projections, f32r) is woven with
  phase B (attention) at emission time so the PE queue never drains: the
  TRN2 PE p-state only reaches 2.4 GHz after ~3us of continuous busy.
- Q^T/K^T stored fp16; logits matmuls fp16 (1 cyc/row at any N, enabling
  exact causal column trimming).
- PV uses stationary=exp-tile chunk [128k,128q], moving=V||ones [128k,129]:
  output lands natural [q, dv] with softmax row-sums free in column 128.
  No ones-matmuls, no [1,512] reciprocals, no LN transpose round trip.
- Diff d = E0V - (lam*s0/s1)*E1V via one fused scalar_tensor_tensor per
  q-subtile (free row-sum accumulation for the LN mean); sum(d^2) via one
  tensor_tensor_reduce. LayerNorm rstd for all 32 tiles computed with ONE
  Ln + ONE Exp (2 ACT table loads total).
- Phase B.5 normalizes + PE-transposes to MT fp16; phase C out-proj in fp16
  with Wo host-cast to fp16 and prefetched.
"""
import math
from contextlib import ExitStack

import numpy as np

import concourse.bass as bass
import concourse.mybir as mybir
import concourse.tile as tile
from concourse import bacc
from concourse.bass_utils import run_bass_kernel_spmd
from concourse.masks import make_identity

F32 = mybir.dt.float32
F32R = mybir.dt.float32r
F16 = mybir.dt.float16
AF = mybir.ActivationFunctionType
OP = mybir.AluOpType

T = 2048
D_EMB = 2048
HD = 64           # head dim per stream
N_CORES = 8
H_LOC = 2         # heads per core
SCALE = HD ** -0.5
LAMBDA_INIT = 0.8 - 0.6 * math.exp(-0.3 * 12)
LN_EPS = 1e-5

TB = 4            # t-blocks of 512 in phase A
QB = 4            # q-blocks of 512 in phase B
KD = 16           # contraction chunks of 128 over D_EMB


def _build_program():
    nc = bacc.Bacc("TRN2", target_bir_lowering=False, debug=False)

    x_d = nc.dram_tensor("x", [T, D_EMB], F32R, kind="ExternalInput").ap()
    wq_d = nc.dram_tensor("wq", [D_EMB, 256], F32R, kind="ExternalInput").ap()
    wk_d = nc.dram_tensor("wk", [D_EMB, 256], F32R, kind="ExternalInput").ap()
    wv_d = nc.dram_tensor("wv", [D_EMB, 256], F32R, kind="ExternalInput").ap()
    wo_d = nc.dram_tensor("wo", [D_EMB, D_EMB], F16, kind="ExternalInput").ap()
    neglam_d = nc.dram_tensor("neglam", [128, 1], F32, kind="ExternalInput").ap()
    y_d = nc.dram_tensor("y", [256, D_EMB], F32, kind="ExternalOutput").ap()

    with tile.TileContext(nc) as tc:
        with ExitStack() as ctx:
            const = ctx.enter_context(tc.tile_pool(name="const", bufs=1))
            qkv = ctx.enter_context(tc.tile_pool(name="qkv", bufs=1))
            dst = ctx.enter_context(tc.tile_pool(name="dst", bufs=1))
            small = ctx.enter_context(tc.tile_pool(name="small", bufs=3))
            etp = ctx.enter_context(tc.tile_pool(name="etp", bufs=6))
            ps = ctx.enter_context(tc.tile_pool(name="ps", bufs=2, space="PSUM"))
            ps_log = ctx.enter_context(
                tc.tile_pool(name="ps_log", bufs=2, space="PSUM"))
            ps_pv0 = ctx.enter_context(
                tc.tile_pool(name="ps_pv0", bufs=2, space="PSUM"))
            ps_pv1 = ctx.enter_context(
                tc.tile_pool(name="ps_pv1", bufs=2, space="PSUM"))

            # ---------------- constants ----------------
            ident = const.tile([128, 128], F32)
            make_identity(nc, ident[:])
            ident_r = const.tile([128, 128], F32R)
            nc.scalar.copy(ident_r[:], ident[:])
            ident_h = const.tile([128, 128], F16)
            nc.scalar.copy(ident_h[:], ident[:])
            neglam = const.tile([128, 1], F32)
            nc.scalar.dma_start(neglam[:], neglam_d)
            eps_t = const.tile([128, 1], F32)
            nc.gpsimd.memset(eps_t[:], LN_EPS)
            lnc_t = const.tile([128, 1], F32)
            nc.gpsimd.memset(lnc_t[:], math.log(1.0 - LAMBDA_INIT))
            ebias_t = const.tile([128, 1], F32)
            nc.gpsimd.memset(ebias_t[:], -6.0)
            # tri[k, q] = 1 where k <= q else 0 (causal keep-mask, fp16)
            tri = const.tile([128, 128], F16)
            nc.gpsimd.memset(tri[:], 1.0)
            nc.gpsimd.affine_select(
                out=tri[:], in_=tri[:], compare_op=OP.is_ge, fill=0.0,
                base=0, pattern=[[1, 128]], channel_multiplier=-1)

            # ---------------- persistent tensors ----------------
            QT = [qkv.tile([128, T], F16, name=f"qt{h}") for h in range(H_LOC)]
            KT = [qkv.tile([128, T], F16, name=f"kt{h}") for h in range(H_LOC)]
            # V[t]: [k(128), head(2), 132]; cols 0:128 = V data, col 128 = 1.0
            V = [qkv.tile([128, 2, 132], F16, name=f"v{t}") for t in range(16)]
            MT = [qkv.tile([128, T], F16, name=f"mt{h}") for h in range(H_LOC)]
            for t in range(16):
                nc.gpsimd.memset(V[t][:, :, 128:129], 1.0)

            # d staging + LN statistics (col = h*16 + qb*4 + qj)
            dtiles = [[dst.tile([128, 128], F16, name=f"d{h}_{i}")
                       for i in range(16)] for h in range(H_LOC)]
            dsums = dst.tile([128, 32], F32, name="dsums")
            sumsq = dst.tile([128, 32], F32, name="sumsq")
            mus = dst.tile([128, 32], F32, name="mus")
            rstd = dst.tile([128, 32], F32, name="rstd")

            # ---------------- phase B closures ----------------
            def gen_b_closures(h, qb):
                nck = 4 * qb + 4
                st_state = {}

                def mk_u1(kc):
                    def u1():
                        if kc == 0:
                            st_state["pv"] = [
                                [ps_pv0.tile([128, 2, 136], F32, name="pv0")
                                 for _ in range(2)],
                                [ps_pv1.tile([128, 2, 136], F32, name="pv1")
                                 for _ in range(2)],
                            ]
                        j = kc - 4 * qb
                        qs = 128 * j if j > 0 else 0
                        ets = []
                        for s in (0, 1):
                            stp = ps_log.tile([128, 512], F32, name="pslog")
                            nc.tensor.matmul(
                                stp[:, qs:512],
                                KT[h][s * 64:(s + 1) * 64,
                                      kc * 128:(kc + 1) * 128],
                                QT[h][s * 64:(s + 1) * 64,
                                      qb * 512 + qs:(qb + 1) * 512],
                                start=True, stop=True)
                            # bias -6 keeps exp and the E*V products in fp16
                            # range; the softmax ratio and LayerNorm are
                            # invariant to the uniform e^-6 factor
                            et = etp.tile([128, 512], F16, name="et")
                            nc.scalar.activation(et[:, qs:512], stp[:, qs:512],
                                                 AF.Exp, scale=SCALE,
                                                 bias=ebias_t[:])
                            if j >= 0:
                                nc.gpsimd.tensor_tensor(
                                    et[:, qs:qs + 128], et[:, qs:qs + 128],
                                    tri[:], OP.mult)
                            ets.append(et)
                        st_state[kc] = ets
                    return u1

                def mk_u2(kc):
                    def u2():
                        j = kc - 4 * qb
                        ets = st_state.pop(kc)
                        pv = st_state["pv"]
                        for s in (0, 1):
                            for qj in range(4):
                                if j > qj:
                                    continue
                                # start=True zeroes the WHOLE 2KB psum zero
                                # region, so only the first matmul into each
                                # bank starts; the odd-qj group's first write
                                # lands on pending-zero bytes and overwrites.
                                nc.tensor.matmul(
                                    pv[s][qj // 2][:, qj % 2, 0:129],
                                    ets[s][:, qj * 128:(qj + 1) * 128],
                                    V[kc][:, h, 0:129],
                                    start=(kc == 0 and qj % 2 == 0),
                                    stop=(kc == 4 * qb + qj),
                                    skip_group_check=True)
                    return u2

                def epi():
                    pv = st_state.pop("pv")
                    for qj in range(4):
                        col = h * 16 + qb * 4 + qj
                        p0 = pv[0][qj // 2][:, qj % 2, :]
                        p1 = pv[1][qj // 2][:, qj % 2, :]
                        # d = p0/s0 - lam*p1/s1 — matching the reference's
                        # softmax normalization exactly (so LN_EPS compares
                        # against the same variance scale, and the exp bias
                        # e^-6 cancels)
                        r1 = small.tile([128, 1], F32, name="r1")
                        nc.vector.reciprocal(r1[:], p1[:, 128:129])
                        r0 = small.tile([128, 1], F32, name="r0")
                        nc.vector.reciprocal(r0[:], p0[:, 128:129])
                        negc = small.tile([128, 1], F32, name="negc")
                        nc.vector.tensor_tensor(
                            negc[:], neglam[:], r1[:], OP.mult)
                        dt_ = dtiles[h][qb * 4 + qj]
                        tmp = small.tile([128, 128], F32, name="tmp")
                        nc.vector.tensor_scalar(
                            tmp[:], p1[:, 0:128], negc[:], None, OP.mult)
                        nc.vector.scalar_tensor_tensor(
                            dt_[:], p0[:, 0:128], r0[:], tmp[:],
                            op0=OP.mult, op1=OP.add,
                            accum_out=dsums[:, col:col + 1])
                        # (d * 1.0) * d with accumulated sum -> sum(d^2);
                        # native InstTensorScalarPtr (tensor_tensor_reduce is
                        # a custom-DVE op whose ucode table crashes this
                        # execution path on hardware)
                        dsq = small.tile([128, 128], F16, name="dsq")
                        nc.vector.scalar_tensor_tensor(
                            dsq[:], dt_[:], 1.0, dt_[:],
                            op0=OP.mult, op1=OP.mult,
                            accum_out=sumsq[:, col:col + 1])

                # lookahead order: u1(k+1) is emitted before u2(k) so the
                # exp of tile k finishes behind the logits matmuls of k+1
                us = [mk_u1(kc) for kc in range(nck)]
                vs = [mk_u2(kc) for kc in range(nck)]
                out = [us[0]]
                for kc in range(1, nck):
                    out.append(us[kc])
                    out.append(vs[kc - 1])
                out.append(vs[nck - 1])
                out.append(epi)
                return out

            def weave(quanta, bcl):
                n, m = len(bcl), max(1, len(quanta))
                bi = 0
                for i, q in enumerate(quanta):
                    q()
                    tgt = (i + 1) * n // m
                    while bi < tgt:
                        bcl[bi]()
                        bi += 1
                while bi < n:
                    bcl[bi]()
                    bi += 1

            # ---------------- phase A (woven with B) ----------------
            with ExitStack() as actx:
                wpool = actx.enter_context(tc.tile_pool(name="wpool", bufs=1))
                xpool = actx.enter_context(tc.tile_pool(name="xpool", bufs=2))
                xtc = actx.enter_context(tc.tile_pool(name="xtc", bufs=16))

                wq_t = wpool.tile([128, KD, 256], F32R, name="wq")
                wk_t = wpool.tile([128, KD, 256], F32R, name="wk")
                wv_t = wpool.tile([128, KD, 256], F32R, name="wv")
                # halves interleaved so early dj chunks of all three arrive
                # first (scalar queue; x tiles go on the sync queue)
                for lo, hi in ((0, 8), (8, 16)):
                    for w_t, w_d in ((wq_t, wq_d), (wk_t, wk_d), (wv_t, wv_d)):
                        nc.scalar.dma_start(
                            w_t[:, lo:hi, :],
                            w_d[lo * 128:hi * 128, :].rearrange(
                                "(a p) c -> p a c", p=128))

                for tb in range(TB):
                    x_t = xpool.tile([128, 4, D_EMB], F32R, name="xin")
                    xts = [xtc.tile([128, 512], F32R, name="xtc")
                           for _ in range(KD)]
                    quanta = []

                    def dma_q(x_t=x_t, tb=tb):
                        for tt in range(4):
                            nc.sync.dma_start(
                                x_t[:, tt, :],
                                x_d[tb * 512 + tt * 128:
                                    tb * 512 + (tt + 1) * 128, :])
                    quanta.append(dma_q)

                    for djp in range(8):
                        def tq(djp=djp, x_t=x_t, xts=xts):
                            for dj in (2 * djp, 2 * djp + 1):
                                pst = ps.tile([128, 512], F32R, name="ps")
                                for tt in range(4):
                                    nc.tensor.transpose(
                                        pst[:, tt * 128:(tt + 1) * 128],
                                        x_t[:, tt, dj * 128:(dj + 1) * 128],
                                        ident_r[:])
                                nc.vector.tensor_copy(xts[dj][:], pst[:])
                        quanta.append(tq)

                    for h in range(H_LOC):
                        for w_t, dstq in ((wq_t, QT), (wk_t, KT)):
                            def qk(w_t=w_t, dstq=dstq, h=h, tb=tb, xts=xts):
                                psq = ps.tile([128, 512], F32, name="ps")
                                for dj in range(KD):
                                    nc.tensor.matmul(
                                        psq[:],
                                        w_t[:, dj, h * 128:(h + 1) * 128],
                                        xts[dj][:],
                                        start=(dj == 0), stop=(dj == KD - 1))
                                nc.scalar.copy(
                                    dstq[h][:, tb * 512:(tb + 1) * 512],
                                    psq[:])
                            quanta.append(qk)

                    for tt in range(4):
                        def vq(tt=tt, tb=tb, xts=xts):
                            psv = ps.tile([128, 256], F32, name="ps")
                            for dj in range(KD):
                                nc.tensor.matmul(
                                    psv[:],
                                    xts[dj][:, tt * 128:(tt + 1) * 128],
                                    wv_t[:, dj, :],
                                    start=(dj == 0), stop=(dj == KD - 1))
                            vt = V[tb * 4 + tt]
                            nc.scalar.copy(
                                vt[:, :, 0:128],
                                psv[:].rearrange("p (h c) -> p h c", h=2))
                            # center V rows over dv: LN(d) is exactly
                            # invariant, but removes the near-constant row
                            # component that otherwise amplifies fp16
                            # rounding ~50x through the 1/sigma of rows
                            # where the two streams nearly cancel
                            vsum = small.tile([128, 2], F32, name="vsum")
                            nc.vector.tensor_reduce(
                                out=vsum[:], in_=vt[:, :, 0:128],
                                axis=mybir.AxisListType.X, op=OP.add)
                            nmean = small.tile([128, 2], F32, name="nmean")
                            nc.vector.tensor_scalar(
                                nmean[:], vsum[:], -1.0 / 128.0, None,
                                OP.mult)
                            for hh in range(H_LOC):
                                nc.vector.tensor_scalar(
                                    vt[:, hh, 0:128], vt[:, hh, 0:128],
                                    nmean[:, hh:hh + 1], None, OP.add)
                        quanta.append(vq)

                    # weave previous q-block's attention into this stage
                    bcl = []
                    if tb >= 1:
                        for h in range(H_LOC):
                            bcl += gen_b_closures(h, tb - 1)
                    weave(quanta, bcl)

            # ---------------- phase B tail: qb=3 + Wo prefetch ----------
            wopool = ctx.enter_context(tc.tile_pool(name="wopool", bufs=2))
            wo_tiles = {}

            def wo_dma(do):
                def f():
                    wo_t = wopool.tile([128, KD, 512], F16, name="wo")
                    nc.sync.dma_start(
                        wo_t[:],
                        wo_d[:, do * 512:(do + 1) * 512].rearrange(
                            "(a p) c -> p a c", p=128))
                    wo_tiles[do] = wo_t
                return f

            tail = gen_b_closures(0, 3)
            tail.insert(2, wo_dma(0))
            tail.insert(len(tail) // 2, wo_dma(1))
            tail += gen_b_closures(1, 3)
            for f in tail:
                f()

            # ---------------- phase B.5: LN + transpose to MT ----------
            musq = dst.tile([128, 32], F32, name="musq")
            varp = dst.tile([128, 32], F32, name="varp")
            nc.vector.tensor_scalar(mus[:], dsums[:], 1.0 / 128.0, None,
                                    OP.mult)
            nc.vector.tensor_tensor(musq[:], mus[:], mus[:], OP.mult)
            nc.vector.scalar_tensor_tensor(
                varp[:], sumsq[:], 1.0 / 128.0, musq[:],
                op0=OP.mult, op1=OP.subtract)
            lnv = dst.tile([128, 32], F32, name="lnv")
            nc.scalar.activation(lnv[:], varp[:], AF.Ln, bias=eps_t[:])
            # rstd' = (1-lambda_init) / sqrt(var+eps) = exp(-.5*lnv + lnc)
            nc.scalar.activation(rstd[:], lnv[:], AF.Exp, scale=-0.5,
                                 bias=lnc_t[:])

            for h in range(H_LOC):
                for qb in range(QB):
                    mt_ps = ps.tile([128, 512], F16, name="ps")
                    for qj in range(4):
                        i = qb * 4 + qj
                        col = h * 16 + i
                        mn = small.tile([128, 128], F16, name="mn")
                        nc.vector.tensor_scalar(
                            mn[:], dtiles[h][i][:], mus[:, col:col + 1],
                            rstd[:, col:col + 1], OP.subtract, OP.mult)
                        nc.tensor.transpose(
                            mt_ps[:, qj * 128:(qj + 1) * 128], mn[:],
                            ident_h[:])
                    nc.vector.tensor_copy(
                        MT[h][:, qb * 512:(qb + 1) * 512], mt_ps[:])

            # ---------------- phase C: out-proj ----------------
            with ExitStack() as cctx:
                ypool = cctx.enter_context(tc.tile_pool(name="ypool", bufs=2))
                mt_r = [MT[h][:].rearrange("p (m g) -> p g m", g=16)
                        for h in range(H_LOC)]
                for do in range(4):
                    if do not in wo_tiles:
                        wo_dma(do)()
                    wo_t = wo_tiles.pop(do)
                    if do + 2 < 4:
                        wo_dma(do + 2)()
                    ys = [ps.tile([128, 512], F32, name="ps")
                          for _ in range(H_LOC)]
                    for j in range(KD):
                        for h in range(H_LOC):
                            nc.tensor.matmul(
                                ys[h][:], mt_r[h][:, j, :], wo_t[:, j, :],
                                start=(j == 0), stop=(j == KD - 1))
                    for h in range(H_LOC):
                        y_sb = ypool.tile([128, 512], F32, name="ysb")
                        nc.scalar.copy(y_sb[:], ys[h][:])
                        nc.scalar.dma_start(
                            y_d[h * 128:(h + 1) * 128,
                                do * 512:(do + 1) * 512],
                            y_sb[:])

    nc.compile()
    return nc


_NC_CACHE = None


def make_in_maps(x, Wq, Wk, Wv, Wo, lambda_q1, lambda_k1, lambda_q2,
                 lambda_k2):
    x2 = np.ascontiguousarray(
        np.asarray(x, np.float32).reshape(T, D_EMB))
    Wq = np.asarray(Wq, np.float32)
    Wk = np.asarray(Wk, np.float32)
    Wv = np.asarray(Wv, np.float32)
    Wo16 = np.ascontiguousarray(np.asarray(Wo, np.float32).astype(np.float16))

    lam = (math.exp(float(np.dot(np.asarray(lambda_q1, np.float64),
                                 np.asarray(lambda_k1, np.float64))))
           - math.exp(float(np.dot(np.asarray(lambda_q2, np.float64),
                                   np.asarray(lambda_k2, np.float64))))
           + LAMBDA_INIT)
    neglam = np.full((128, 1), -lam, dtype=np.float32)

    in_maps = []
    for m in range(N_CORES):
        sl = slice(256 * m, 256 * (m + 1))
        in_maps.append({
            "x": x2,
            "wq": np.ascontiguousarray(Wq[:, sl]),
            "wk": np.ascontiguousarray(Wk[:, sl]),
            "wv": np.ascontiguousarray(Wv[:, sl]),
            "wo": Wo16,
            "neglam": neglam,
        })
    return in_maps


def kernel(x, Wq, Wk, Wv, Wo, lambda_q1, lambda_k1, lambda_q2, lambda_k2):
    global _NC_CACHE
    if _NC_CACHE is None:
        _NC_CACHE = _build_program()
    nc = _NC_CACHE

    B = np.asarray(x).shape[0]
    in_maps = make_in_maps(x, Wq, Wk, Wv, Wo, lambda_q1, lambda_k1,
                           lambda_q2, lambda_k2)
    res = run_bass_kernel_spmd(nc, in_maps, list(range(N_CORES)))
    y = np.concatenate([res.results[m]["y"] for m in range(N_CORES)], axis=0)
    return y.reshape(B, T, D_EMB)



# revision 7
# speedup vs baseline: 1.0074x; 1.0074x over previous
"""DiffAttn (Differential Transformer attention) on 8 trn2 NeuronCores.

Sharding: tensor-parallel over heads. 16 heads / 8 cores = 2 heads per core.
Wq/Wk/Wv column-sharded (256 cols/core), Wo + x replicated. The reference's
"reshape without transposing heads back" maps output row r = h*128 + (t//16)
entirely to head h, so each core produces rows [256*m, 256*(m+1)) of the
final (2048, 2048) output with NO collectives.

v3 design notes (perf, from the v2 NTFF trace):
- v2 spent 97us at the HAM K=4/8 half-clock state: ~47us at the start
  (PE idle waiting on x DMA + slow ramp) and ~82us in the serial qb=3
  tail (logits->exp->mask->PV chain at lookahead-1 = 2.8us/step for
  1.25us of PE work; duty too low for HAM to re-promote).
- fp32 LDWEIGHTS (no FWL) cost 76.6us. v3 runs the whole QKV pipeline in
  fp16 (host-cast W, x^T cast to fp16 in the PSUM->SBUF copy): fp16 LDW
  gets FWL and hides behind the matmul streams.
- v3 start: ~40 warmup transposes trip HAM to 8/8 immediately; x is
  DMA'd in [128,2048] chunks with per-chunk transpose quanta so the PE
  has real work ~4us in; x(tb+1) chunks prefetch during tb.
- Tail: h0's qb=3 chain runs at kc-lookahead-2 (ps_log bufs=4), then
  B.5(h0) + out-proj(h0) weave INTO h1's qb=3 chain as PE filler.
- LN: mean over dv is exactly 0 by V-centering (softmax rows sum to 1),
  so mean subtraction is dropped; rstd folds into the B.5 transpose as
  a diagonal rhs (out = d.T @ diag(rstd)) - no separate normalize pass.
- Weight/Wo/y DMAs moved off the scalar queue (they occupied ACT for
  ~36us between exps in v2).
"""
import math
from contextlib import ExitStack

import numpy as np

import concourse.bass as bass
import concourse.mybir as mybir
import concourse.tile as tile
from concourse import bacc
from concourse.bass_utils import run_bass_kernel_spmd
from concourse.masks import make_identity

F32 = mybir.dt.float32
F32R = mybir.dt.float32r
F16 = mybir.dt.float16
AF = mybir.ActivationFunctionType
OP = mybir.AluOpType

T = 2048
D_EMB = 2048
HD = 64           # head dim per stream
N_CORES = 8
H_LOC = 2         # heads per core
SCALE = HD ** -0.5
LAMBDA_INIT = 0.8 - 0.6 * math.exp(-0.3 * 12)
LN_EPS = 1e-5

TB = 4            # t-blocks of 512 in phase A
QB = 4            # q-blocks of 512 in phase B
KD = 16           # contraction chunks of 128 over D_EMB


def _build_program():
    nc = bacc.Bacc("TRN2", target_bir_lowering=False, debug=False)

    x_d = nc.dram_tensor("x", [T, D_EMB], F32R, kind="ExternalInput").ap()
    wq_d = nc.dram_tensor("wq", [D_EMB, 256], F16, kind="ExternalInput").ap()
    wk_d = nc.dram_tensor("wk", [D_EMB, 256], F16, kind="ExternalInput").ap()
    wv_d = nc.dram_tensor("wv", [D_EMB, 256], F16, kind="ExternalInput").ap()
    wo_d = nc.dram_tensor("wo", [D_EMB, D_EMB], F16, kind="ExternalInput").ap()
    neglam_d = nc.dram_tensor("neglam", [128, 1], F32, kind="ExternalInput").ap()
    y_d = nc.dram_tensor("y", [256, D_EMB], F32, kind="ExternalOutput").ap()

    with tile.TileContext(nc) as tc:
        with ExitStack() as ctx:
            const = ctx.enter_context(tc.tile_pool(name="const", bufs=1))
            qkv = ctx.enter_context(tc.tile_pool(name="qkv", bufs=1))
            dst = ctx.enter_context(tc.tile_pool(name="dst", bufs=1))
            small = ctx.enter_context(tc.tile_pool(name="small", bufs=4))
            ypool = ctx.enter_context(tc.tile_pool(name="ypool", bufs=2))
            etp = ctx.enter_context(tc.tile_pool(name="etp", bufs=6))
            ps_pv0 = ctx.enter_context(
                tc.tile_pool(name="ps_pv0", bufs=2, space="PSUM"))
            ps_pv1 = ctx.enter_context(
                tc.tile_pool(name="ps_pv1", bufs=2, space="PSUM"))

            # ---------------- constants ----------------
            ident = const.tile([128, 128], F32)
            make_identity(nc, ident[:])
            ident_r = const.tile([128, 128], F32R)
            nc.scalar.copy(ident_r[:], ident[:])
            ident_h = const.tile([128, 128], F16)
            nc.scalar.copy(ident_h[:], ident[:])
            neglam = const.tile([128, 1], F32)
            nc.scalar.dma_start(neglam[:], neglam_d)
            eps_t = const.tile([128, 1], F32)
            nc.gpsimd.memset(eps_t[:], LN_EPS)
            lnc_t = const.tile([128, 1], F32)
            nc.gpsimd.memset(lnc_t[:], math.log(1.0 - LAMBDA_INIT))
            ebias_t = const.tile([128, 1], F32)
            nc.gpsimd.memset(ebias_t[:], -6.0)
            # tri[k, q] = 1 where k <= q else 0 (causal keep-mask, fp16)
            tri = const.tile([128, 128], F16)
            nc.gpsimd.memset(tri[:], 1.0)
            nc.gpsimd.affine_select(
                out=tri[:], in_=tri[:], compare_op=OP.is_ge, fill=0.0,
                base=0, pattern=[[1, 128]], channel_multiplier=-1)

            # ---------------- persistent tensors ----------------
            QT = [qkv.tile([128, T], F16, name=f"qt{h}") for h in range(H_LOC)]
            KT = [qkv.tile([128, T], F16, name=f"kt{h}") for h in range(H_LOC)]
            # V[t]: [k(128), head(2), 132]; cols 0:128 = V data, col 128 = 1.0
            V = [qkv.tile([128, 2, 132], F16, name=f"v{t}") for t in range(16)]
            MT = [qkv.tile([128, T], F16, name=f"mt{h}") for h in range(H_LOC)]
            for t in range(16):
                nc.gpsimd.memset(V[t][:, :, 128:129], 1.0)

            # d staging + LN statistics (col = h*16 + qb*4 + qj)
            dtiles = [[dst.tile([128, 128], F16, name=f"d{h}_{i}")
                       for i in range(16)] for h in range(H_LOC)]
            sumsq = dst.tile([128, 32], F32, name="sumsq")
            varp = dst.tile([128, 32], F32, name="varp")
            lnv = dst.tile([128, 32], F32, name="lnv")
            rstd = dst.tile([128, 32], F32, name="rstd")

            # ---------------- phase B closures ----------------
            def gen_b_closures(h, qb, ps_log, lookahead=1):
                nck = 4 * qb + 4
                st_state = {}

                def mk_u1(kc):
                    def u1():
                        if kc == 0:
                            st_state["pv"] = [
                                [ps_pv0.tile([128, 2, 136], F32, name="pv0")
                                 for _ in range(2)],
                                [ps_pv1.tile([128, 2, 136], F32, name="pv1")
                                 for _ in range(2)],
                            ]
                        j = kc - 4 * qb
                        qs = 128 * j if j > 0 else 0
                        ets = []
                        for s in (0, 1):
                            stp = ps_log.tile([128, 512], F32, name="pslog")
                            nc.tensor.matmul(
                                stp[:, qs:512],
                                KT[h][s * 64:(s + 1) * 64,
                                      kc * 128:(kc + 1) * 128],
                                QT[h][s * 64:(s + 1) * 64,
                                      qb * 512 + qs:(qb + 1) * 512],
                                start=True, stop=True)
                            # bias -6 keeps exp and the E*V products in fp16
                            # range; the softmax ratio and LayerNorm are
                            # invariant to the uniform e^-6 factor
                            et = etp.tile([128, 512], F16, name="et")
                            nc.scalar.activation(et[:, qs:512], stp[:, qs:512],
                                                 AF.Exp, scale=SCALE,
                                                 bias=ebias_t[:])
                            if j >= 0:
                                nc.gpsimd.tensor_tensor(
                                    et[:, qs:qs + 128], et[:, qs:qs + 128],
                                    tri[:], OP.mult)
                            ets.append(et)
                        st_state[kc] = ets
                    return u1

                def mk_u2(kc):
                    def u2():
                        j = kc - 4 * qb
                        ets = st_state.pop(kc)
                        pv = st_state["pv"]
                        for s in (0, 1):
                            for qj in range(4):
                                if j > qj:
                                    continue
                                # start=True zeroes the WHOLE 2KB psum zero
                                # region, so only the first matmul into each
                                # bank starts; the odd-qj group's first write
                                # lands on pending-zero bytes and overwrites.
                                nc.tensor.matmul(
                                    pv[s][qj // 2][:, qj % 2, 0:129],
                                    ets[s][:, qj * 128:(qj + 1) * 128],
                                    V[kc][:, h, 0:129],
                                    start=(kc == 0 and qj % 2 == 0),
                                    stop=(kc == 4 * qb + qj),
                                    skip_group_check=True)
                    return u2

                def epi():
                    pv = st_state.pop("pv")
                    for qj in range(4):
                        col = h * 16 + qb * 4 + qj
                        p0 = pv[0][qj // 2][:, qj % 2, :]
                        p1 = pv[1][qj // 2][:, qj % 2, :]
                        # d = p0/s0 - lam*p1/s1 — matching the reference's
                        # softmax normalization exactly (so LN_EPS compares
                        # against the same variance scale, and the exp bias
                        # e^-6 cancels)
                        r1 = small.tile([128, 1], F32, name="r1")
                        nc.vector.reciprocal(r1[:], p1[:, 128:129])
                        r0 = small.tile([128, 1], F32, name="r0")
                        nc.vector.reciprocal(r0[:], p0[:, 128:129])
                        negc = small.tile([128, 1], F32, name="negc")
                        nc.vector.tensor_tensor(
                            negc[:], neglam[:], r1[:], OP.mult)
                        dt_ = dtiles[h][qb * 4 + qj]
                        tmp = small.tile([128, 128], F32, name="tmp")
                        nc.vector.tensor_scalar(
                            tmp[:], p1[:, 0:128], negc[:], None, OP.mult)
                        nc.vector.scalar_tensor_tensor(
                            dt_[:], p0[:, 0:128], r0[:], tmp[:],
                            op0=OP.mult, op1=OP.add)
                        # (d * 1.0) * d with accumulated sum -> sum(d^2);
                        # native InstTensorScalarPtr (tensor_tensor_reduce is
                        # a custom-DVE op whose ucode table crashes this
                        # execution path on hardware)
                        dsq = small.tile([128, 128], F16, name="dsq")
                        nc.vector.scalar_tensor_tensor(
                            dsq[:], dt_[:], 1.0, dt_[:],
                            op0=OP.mult, op1=OP.mult,
                            accum_out=sumsq[:, col:col + 1])

                # lookahead order: u1(k+L) is emitted before u2(k) so the
                # exp/mask of tile k finishes behind later logits matmuls
                us = [mk_u1(kc) for kc in range(nck)]
                vs = [mk_u2(kc) for kc in range(nck)]
                out = []
                la = min(lookahead, nck)
                for kc in range(la):
                    out.append(us[kc])
                for kc in range(la, nck):
                    out.append(us[kc])
                    out.append(vs[kc - la])
                for kc in range(nck - la, nck):
                    out.append(vs[kc])
                out.append(epi)
                return out

            def weave(quanta, bcl):
                n, m = len(bcl), max(1, len(quanta))
                bi = 0
                for i, q in enumerate(quanta):
                    q()
                    tgt = (i + 1) * n // m
                    while bi < tgt:
                        bcl[bi]()
                        bi += 1
                while bi < n:
                    bcl[bi]()
                    bi += 1

            # rstd for one head's 16 columns:
            # rstd' = (1-lambda_init)/sqrt(var+eps) = exp(-.5*ln(var+eps)+lnc)
            # (mean over dv is exactly 0 by V-centering + softmax row-sum=1,
            # so var = sumsq/128 with no mu^2 correction)
            def rstd_head(h):
                def f():
                    sl = slice(h * 16, h * 16 + 16)
                    nc.vector.tensor_scalar(
                        varp[:, sl], sumsq[:, sl], 1.0 / 128.0, None, OP.mult)
                    nc.scalar.activation(lnv[:, sl], varp[:, sl], AF.Ln,
                                         bias=eps_t[:])
                    nc.scalar.activation(rstd[:, sl], lnv[:, sl], AF.Exp,
                                         scale=-0.5, bias=lnc_t[:])
                return f

            # B.5 for one (h, qb): transpose d*rstd into MT via diag rhs
            def gen_b5_quanta(h, ps_c):
                quanta = []
                for qb in range(QB):
                    def b5q(h=h, qb=qb):
                        mt_ps = ps_c.tile([128, 512], F32, name="psc")
                        for qj in range(4):
                            i = qb * 4 + qj
                            col = h * 16 + i
                            diag = small.tile([128, 128], F16, name="diag")
                            nc.vector.tensor_scalar(
                                diag[:], ident_h[:], rstd[:, col:col + 1],
                                None, OP.mult)
                            # regular matmul, NOT transpose-mode: the
                            # transpose datapath is a permutation that
                            # ignores the rhs, so d.T @ diag(rstd) must go
                            # through the MAC array
                            nc.tensor.matmul(
                                mt_ps[:, qj * 128:(qj + 1) * 128],
                                dtiles[h][i][:], diag[:],
                                start=True, stop=True)
                        nc.vector.tensor_copy(
                            MT[h][:, qb * 512:(qb + 1) * 512], mt_ps[:])
                    quanta.append(b5q)
                return quanta

            # ---------------- phase A (woven with B) ----------------
            wopool = None
            with ExitStack() as actx:
                wpool = actx.enter_context(tc.tile_pool(name="wpool", bufs=1))
                xchunk = actx.enter_context(
                    tc.tile_pool(name="xchunk", bufs=5))
                xtsp = actx.enter_context(tc.tile_pool(name="xtsp", bufs=2))
                ps_a = actx.enter_context(
                    tc.tile_pool(name="ps_a", bufs=2, space="PSUM"))
                ps_log_a = actx.enter_context(
                    tc.tile_pool(name="ps_log_a", bufs=2, space="PSUM"))

                # PE warmup: ~40 back-to-back transposes trip the HAM clock
                # gate to 8/8 before the first x chunk lands
                wps = ps_a.tile([128, 512], F32R, name="ps")
                for i in range(40):
                    nc.tensor.transpose(
                        wps[:, (i % 4) * 128:(i % 4 + 1) * 128],
                        ident_r[:], ident_r[:])

                wq_t = wpool.tile([128, KD, 256], F16, name="wq")
                wk_t = wpool.tile([128, KD, 256], F16, name="wk")
                wv_t = wpool.tile([128, KD, 256], F16, name="wv")
                # halves interleaved so early dj chunks of all three arrive
                # first; gpsimd queue (scalar queue carries exps in phase B)
                for lo, hi in ((0, 8), (8, 16)):
                    for w_t, w_d in ((wq_t, wq_d), (wk_t, wk_d), (wv_t, wv_d)):
                        nc.gpsimd.dma_start(
                            w_t[:, lo:hi, :],
                            w_d[lo * 128:hi * 128, :].rearrange(
                                "(a p) c -> p a c", p=128))

                xts = [None, None]

                def xq(tb, tt):
                    ch = xchunk.tile([128, D_EMB], F32R, name="xch")
                    nc.sync.dma_start(
                        ch[:],
                        x_d[tb * 512 + tt * 128:tb * 512 + (tt + 1) * 128, :])
                    return ch

                # first t-block's chunks: first DMAs on the sync queue
                chunks = {(0, tt): xq(0, tt) for tt in range(4)}

                for tb in range(TB):
                    xts[tb % 2] = xtsp.tile([128, KD, 512], F16, name="xts")
                    xt = xts[tb % 2]
                    quanta = []

                    for tt in range(4):
                        for g in range(4):  # groups of 4 dj chunks
                            def tq(tt=tt, g=g, tb=tb, xt=xt):
                                ch = chunks[(tb, tt)]
                                pst = ps_a.tile([128, 512], F32R, name="ps")
                                for k in range(4):
                                    dj = g * 4 + k
                                    nc.tensor.transpose(
                                        pst[:, k * 128:(k + 1) * 128],
                                        ch[:, dj * 128:(dj + 1) * 128],
                                        ident_r[:])
                                nc.vector.tensor_copy(
                                    xt[:, g * 4:g * 4 + 4,
                                       tt * 128:(tt + 1) * 128],
                                    pst[:].rearrange("p (a c) -> p a c", a=4))
                            quanta.append(tq)
                        if tb + 1 < TB:
                            def pf(tb=tb, tt=tt):
                                chunks[(tb + 1, tt)] = xq(tb + 1, tt)
                            quanta.append(pf)

                    for h in range(H_LOC):
                        for w_t, dstq in ((wq_t, QT), (wk_t, KT)):
                            def qk(w_t=w_t, dstq=dstq, h=h, tb=tb, xt=xt):
                                psq = ps_a.tile([128, 512], F32, name="ps")
                                for dj in range(KD):
                                    nc.tensor.matmul(
                                        psq[:],
                                        w_t[:, dj, h * 128:(h + 1) * 128],
                                        xt[:, dj, :],
                                        start=(dj == 0), stop=(dj == KD - 1))
                                nc.vector.tensor_copy(
                                    dstq[h][:, tb * 512:(tb + 1) * 512],
                                    psq[:])
                            quanta.append(qk)

                    for tt in range(4):
                        def vq(tt=tt, tb=tb, xt=xt):
                            psv = ps_a.tile([128, 256], F32, name="ps")
                            for dj in range(KD):
                                nc.tensor.matmul(
                                    psv[:],
                                    xt[:, dj, tt * 128:(tt + 1) * 128],
                                    wv_t[:, dj, :],
                                    start=(dj == 0), stop=(dj == KD - 1))
                            vt = V[tb * 4 + tt]
                            nc.scalar.copy(
                                vt[:, :, 0:128],
                                psv[:].rearrange("p (h c) -> p h c", h=2))
                            # center V rows over dv: LN(d) is exactly
                            # invariant (softmax rows sum to 1), removes the
                            # near-constant row component that otherwise
                            # amplifies fp16 rounding ~50x through 1/sigma of
                            # rows where the two streams nearly cancel, and
                            # makes mean_dv(d) exactly 0 so B.5 skips mu
                            vsum = small.tile([128, 2], F32, name="vsum")
                            nc.vector.tensor_reduce(
                                out=vsum[:], in_=vt[:, :, 0:128],
                                axis=mybir.AxisListType.X, op=OP.add)
                            nmean = small.tile([128, 2], F32, name="nmean")
                            nc.vector.tensor_scalar(
                                nmean[:], vsum[:], -1.0 / 128.0, None,
                                OP.mult)
                            for hh in range(H_LOC):
                                nc.vector.tensor_scalar(
                                    vt[:, hh, 0:128], vt[:, hh, 0:128],
                                    nmean[:, hh:hh + 1], None, OP.add)
                        quanta.append(vq)

                    # weave previous q-block's attention into this stage
                    bcl = []
                    if tb >= 1:
                        for h in range(H_LOC):
                            bcl += gen_b_closures(h, tb - 1, ps_log_a)
                    weave(quanta, bcl)

            # ---------------- tail: qb=3 for both heads ----------------
            wopool = ctx.enter_context(tc.tile_pool(name="wopool", bufs=2))
            wo_tiles = {}

            def wo_dma(do):
                def f():
                    wo_t = wopool.tile([128, KD, 512], F16, name="wo")
                    nc.sync.dma_start(
                        wo_t[:],
                        wo_d[:, do * 512:(do + 1) * 512].rearrange(
                            "(a p) c -> p a c", p=128))
                    wo_tiles[do] = wo_t
                return f

            def gen_c_quanta(h, dos, ps_c):
                quanta = []
                for do in dos:
                    def cq(h=h, do=do):
                        wo_t = wo_tiles[do]
                        ys = ps_c.tile([128, 512], F32, name="psc")
                        mt_r = MT[h][:].rearrange("p (m g) -> p g m", g=16)
                        for j in range(KD):
                            nc.tensor.matmul(
                                ys[:], mt_r[:, j, :], wo_t[:, j, :],
                                start=(j == 0), stop=(j == KD - 1))
                        y_sb = ypool.tile([128, 512], F32, name="ysb")
                        nc.vector.tensor_copy(y_sb[:], ys[:])
                        nc.sync.dma_start(
                            y_d[h * 128:(h + 1) * 128,
                                do * 512:(do + 1) * 512],
                            y_sb[:])
                    quanta.append(cq)
                return quanta

            # h0 tail at kc-lookahead-2 (ps_log bufs=4); Wo halves 0,1
            # prefetch on the sync queue behind it
            with ExitStack() as t0ctx:
                ps_log_t0 = t0ctx.enter_context(
                    tc.tile_pool(name="ps_log_t0", bufs=4, space="PSUM"))
                tail0 = gen_b_closures(0, 3, ps_log_t0, lookahead=2)
                tail0.insert(2, wo_dma(0))
                tail0.insert(len(tail0) // 2, wo_dma(1))
                for f in tail0:
                    f()

            # h1 tail woven with B.5(h0) + out-proj(h0)
            with ExitStack() as t1ctx:
                ps_log_t1 = t1ctx.enter_context(
                    tc.tile_pool(name="ps_log_t1", bufs=3, space="PSUM"))
                ps_c1 = t1ctx.enter_context(
                    tc.tile_pool(name="ps_c1", bufs=1, space="PSUM"))
                tail1 = gen_b_closures(1, 3, ps_log_t1, lookahead=2)
                bcl = [rstd_head(0)]
                bcl += gen_b5_quanta(0, ps_c1)
                c0 = gen_c_quanta(0, [0, 1, 2, 3], ps_c1)
                bcl += [c0[0], c0[1], wo_dma(2), c0[2], wo_dma(3), c0[3]]
                weave(tail1, bcl)

            # ---------------- B.5(h1) + out-proj(h1) ----------------
            with ExitStack() as cctx:
                ps_cf = cctx.enter_context(
                    tc.tile_pool(name="ps_cf", bufs=2, space="PSUM"))
                rstd_head(1)()
                for q in gen_b5_quanta(1, ps_cf):
                    q()
                # wo halves 2,3 still resident; re-DMA 1,0 behind them
                c1 = gen_c_quanta(1, [2, 3, 1, 0], ps_cf)
                c1[0]()
                c1[1]()
                wo_dma(1)()
                c1[2]()
                wo_dma(0)()
                c1[3]()

    nc.compile()
    return nc


_NC_CACHE = None


def make_in_maps(x, Wq, Wk, Wv, Wo, lambda_q1, lambda_k1, lambda_q2,
                 lambda_k2):
    x2 = np.ascontiguousarray(
        np.asarray(x, np.float32).reshape(T, D_EMB))
    Wq16 = np.asarray(Wq, np.float32).astype(np.float16)
    Wk16 = np.asarray(Wk, np.float32).astype(np.float16)
    Wv16 = np.asarray(Wv, np.float32).astype(np.float16)
    Wo16 = np.ascontiguousarray(np.asarray(Wo, np.float32).astype(np.float16))

    lam = (math.exp(float(np.dot(np.asarray(lambda_q1, np.float64),
                                 np.asarray(lambda_k1, np.float64))))
           - math.exp(float(np.dot(np.asarray(lambda_q2, np.float64),
                                   np.asarray(lambda_k2, np.float64))))
           + LAMBDA_INIT)
    neglam = np.full((128, 1), -lam, dtype=np.float32)

    in_maps = []
    for m in range(N_CORES):
        sl = slice(256 * m, 256 * (m + 1))
        in_maps.append({
            "x": x2,
            "wq": np.ascontiguousarray(Wq16[:, sl]),
            "wk": np.ascontiguousarray(Wk16[:, sl]),
            "wv": np.ascontiguousarray(Wv16[:, sl]),
            "wo": Wo16,
            "neglam": neglam,
        })
    return in_maps


def kernel(x, Wq, Wk, Wv, Wo, lambda_q1, lambda_k1, lambda_q2, lambda_k2):
    global _NC_CACHE
    if _NC_CACHE is None:
        _NC_CACHE = _build_program()
    nc = _NC_CACHE

    B = np.asarray(x).shape[0]
    in_maps = make_in_maps(x, Wq, Wk, Wv, Wo, lambda_q1, lambda_k1,
                           lambda_q2, lambda_k2)
    res = run_bass_kernel_spmd(nc, in_maps, list(range(N_CORES)))
    y = np.concatenate([res.results[m]["y"] for m in range(N_CORES)], axis=0)
    return y.reshape(B, T, D_EMB)


# revision 22
# speedup vs baseline: 1.0791x; 1.0712x over previous
"""DiffAttn (Differential Transformer attention) on 8 trn2 NeuronCores.

Sharding: tensor-parallel over heads. 16 heads / 8 cores = 2 heads per core.
Wq/Wk/Wv column-sharded (256 cols/core), Wo + x replicated. The reference's
"reshape without transposing heads back" maps output row r = h*128 + (t//16)
entirely to head h, so each core produces rows [256*m, 256*(m+1)) of the
final (2048, 2048) output with NO collectives.

v4 design notes (perf, from v2/v3 NTFF traces):
- The HAM clock gate needs ~3.4us of CONTINUOUS PE busy to reach the
  2.4GHz state and drops back after any ~3.4us lull. v3 lost ~48us to
  half-clock: the serial qb=3 tail (33 kc-step chains with a ~2.2us
  latency chain per step) never re-promoted.
- v4 splits each (h, qb) attention chain at its diagonal: the early
  segment (kc < 4qb) only needs K/V blocks < qb, so it weaves into
  t-block qb's OWN quanta (right after that block's QT/KT are formed);
  the 4 diagonal steps weave into t-block qb+1. The "tail" shrinks to
  the two heads' qb=3 diagonals (interleaved, lookahead via zip),
  epilogues, LN-fold, and the out-projection.
- fp16 everywhere post-transpose (host-cast W, x^T cast in the
  PSUM->SBUF copy): fp16 LDWEIGHTS gets FWL and hides behind streams;
  fp32 LDW (150ns, unhidable) cost v2 77us.
- Start: ~56 warmup transposes trip HAM immediately; x arrives in
  [128,2048] chunks with per-chunk transpose quanta + dummy-transpose
  filler covering the chunk-DMA gaps of the first t-block.
- LN: mean over dv is exactly 0 by V-centering (softmax rows sum to 1)
  so mu is dropped; rstd folds into the d^T transpose as a REAL matmul
  with a diag(rstd) rhs (transpose-mode ignores its rhs).
- Wo fully resident (bufs=4) so both heads' out-proj share each wo_t
  with zero re-DMA; weight/Wo/y DMAs keep off the scalar queue (ACT
  runs the exp chain).
"""
import math
from contextlib import ExitStack

import numpy as np

import concourse.bass as bass
import concourse.mybir as mybir
import concourse.tile as tile
from concourse import bacc
from concourse.bass_utils import run_bass_kernel_spmd
from concourse.masks import make_identity

F32 = mybir.dt.float32
F32R = mybir.dt.float32r
F16 = mybir.dt.float16
AF = mybir.ActivationFunctionType
OP = mybir.AluOpType

T = 2048
D_EMB = 2048
HD = 64           # head dim per stream
N_CORES = 8
H_LOC = 2         # heads per core
SCALE = HD ** -0.5
LAMBDA_INIT = 0.8 - 0.6 * math.exp(-0.3 * 12)
LN_EPS = 1e-5

TB = 4            # t-blocks of 512 in phase A
QB = 4            # q-blocks of 512 in phase B
KD = 16           # contraction chunks of 128 over D_EMB


def _build_program():
    nc = bacc.Bacc("TRN2", target_bir_lowering=False, debug=False)

    x_d = nc.dram_tensor("x", [T, D_EMB], F32R, kind="ExternalInput").ap()
    wq_d = nc.dram_tensor("wq", [D_EMB, 256], F16, kind="ExternalInput").ap()
    wk_d = nc.dram_tensor("wk", [D_EMB, 256], F16, kind="ExternalInput").ap()
    wv_d = nc.dram_tensor("wv", [D_EMB, 256], F16, kind="ExternalInput").ap()
    wo_d = nc.dram_tensor("wo", [D_EMB, D_EMB], F16, kind="ExternalInput").ap()
    neglam_d = nc.dram_tensor("neglam", [128, 1], F32, kind="ExternalInput").ap()
    y_d = nc.dram_tensor("y", [256, D_EMB], F32, kind="ExternalOutput").ap()

    with tile.TileContext(nc) as tc:
        with ExitStack() as ctx:
            const = ctx.enter_context(tc.tile_pool(name="const", bufs=1))
            qkv = ctx.enter_context(tc.tile_pool(name="qkv", bufs=1))
            dst = ctx.enter_context(tc.tile_pool(name="dst", bufs=1))
            small = ctx.enter_context(tc.tile_pool(name="small", bufs=4))
            ypool = ctx.enter_context(tc.tile_pool(name="ypool", bufs=2))
            etp = ctx.enter_context(tc.tile_pool(name="etp", bufs=6))
            ps_pv0 = ctx.enter_context(
                tc.tile_pool(name="ps_pv0", bufs=2, space="PSUM"))
            ps_pv1 = ctx.enter_context(
                tc.tile_pool(name="ps_pv1", bufs=2, space="PSUM"))

            # ---------------- constants ----------------
            ident = const.tile([128, 128], F32)
            make_identity(nc, ident[:])
            ident_r = const.tile([128, 128], F32R)
            nc.scalar.copy(ident_r[:], ident[:])
            ident_h = const.tile([128, 128], F16)
            nc.scalar.copy(ident_h[:], ident[:])
            neglam = const.tile([128, 1], F32)
            nc.scalar.dma_start(neglam[:], neglam_d)
            eps_t = const.tile([128, 1], F32)
            nc.gpsimd.memset(eps_t[:], LN_EPS)
            lnc_t = const.tile([128, 1], F32)
            nc.gpsimd.memset(lnc_t[:], math.log(1.0 - LAMBDA_INIT))
            ebias_t = const.tile([128, 1], F32)
            nc.gpsimd.memset(ebias_t[:], -6.0)
            # tri[k, q] = 1 where k <= q else 0 (causal keep-mask, fp16)
            tri = const.tile([128, 128], F16)
            nc.gpsimd.memset(tri[:], 1.0)
            nc.gpsimd.affine_select(
                out=tri[:], in_=tri[:], compare_op=OP.is_ge, fill=0.0,
                base=0, pattern=[[1, 128]], channel_multiplier=-1)

            # ---------------- persistent tensors ----------------
            QT = [qkv.tile([128, T], F16, name=f"qt{h}") for h in range(H_LOC)]
            KT = [qkv.tile([128, T], F16, name=f"kt{h}") for h in range(H_LOC)]
            # V[t]: [k(128), head(2), 132]; cols 0:128 = V data, col 128 = 1.0
            V = [qkv.tile([128, 2, 132], F16, name=f"v{t}") for t in range(16)]
            MT = [qkv.tile([128, T], F16, name=f"mt{h}") for h in range(H_LOC)]
            for t in range(16):
                nc.gpsimd.memset(V[t][:, :, 128:129], 1.0)

            # d staging + LN statistics (col = h*16 + qb*4 + qj)
            dtiles = [[dst.tile([128, 128], F16, name=f"d{h}_{i}")
                       for i in range(16)] for h in range(H_LOC)]
            sumsq = dst.tile([128, 32], F32, name="sumsq")
            varp = dst.tile([128, 32], F32, name="varp")
            lnv = dst.tile([128, 32], F32, name="lnv")
            rstd = dst.tile([128, 32], F32, name="rstd")

            # ---------------- phase B chain generator ----------------
            class BGen:
                """One (h, qb) attention chain, split at the diagonal.

                early(): closures for kc in [0, 4qb) at lookahead-1
                         (only needs K/V t-blocks < qb).
                diag():  closures for kc in [4qb, 4qb+4), the trailing
                         PV of the early segment, and the 4 per-qj
                         epilogues.
                """

                def __init__(self, h, qb):
                    self.h, self.qb = h, qb
                    self.nck = 4 * qb + 4
                    self.st = {}

                def mk_u1(self, kc, ps_log):
                    h, qb = self.h, self.qb

                    def u1():
                        if kc == 0:
                            self.st["pv"] = [
                                [ps_pv0.tile([128, 2, 136], F32, name="pv0")
                                 for _ in range(2)],
                                [ps_pv1.tile([128, 2, 136], F32, name="pv1")
                                 for _ in range(2)],
                            ]
                        j = kc - 4 * qb
                        qs = 128 * j if j > 0 else 0
                        ets = []
                        for s in (0, 1):
                            stp = ps_log.tile([128, 512], F32, name="pslog")
                            nc.tensor.matmul(
                                stp[:, qs:512],
                                KT[h][s * 64:(s + 1) * 64,
                                      kc * 128:(kc + 1) * 128],
                                QT[h][s * 64:(s + 1) * 64,
                                      qb * 512 + qs:(qb + 1) * 512],
                                start=True, stop=True)
                            # bias -6 keeps exp and the E*V products in fp16
                            # range; the softmax ratio and LayerNorm are
                            # invariant to the uniform e^-6 factor
                            et = etp.tile([128, 512], F16, name="et")
                            nc.scalar.activation(et[:, qs:512], stp[:, qs:512],
                                                 AF.Exp, scale=SCALE,
                                                 bias=ebias_t[:])
                            if j >= 0:
                                nc.gpsimd.tensor_tensor(
                                    et[:, qs:qs + 128], et[:, qs:qs + 128],
                                    tri[:], OP.mult)
                            ets.append(et)
                        self.st[kc] = ets
                    return u1

                def mk_u2(self, kc):
                    h, qb = self.h, self.qb

                    def u2():
                        j = kc - 4 * qb
                        ets = self.st.pop(kc)
                        pv = self.st["pv"]
                        for s in (0, 1):
                            for qj in range(4):
                                if j > qj:
                                    continue
                                # start=True zeroes the WHOLE 2KB psum zero
                                # region, so only the first matmul into each
                                # bank starts; the odd-qj group's first write
                                # lands on pending-zero bytes and overwrites.
                                nc.tensor.matmul(
                                    pv[s][qj // 2][:, qj % 2, 0:129],
                                    ets[s][:, qj * 128:(qj + 1) * 128],
                                    V[kc][:, h, 0:129],
                                    start=(kc == 0 and qj % 2 == 0),
                                    stop=(kc == 4 * qb + qj),
                                    skip_group_check=True)
                    return u2

                def mk_epi(self, qj):
                    h, qb = self.h, self.qb

                    def epi():
                        pv = self.st["pv"] if qj < 3 else self.st.pop("pv")
                        col = h * 16 + qb * 4 + qj
                        p0 = pv[0][qj // 2][:, qj % 2, :]
                        p1 = pv[1][qj // 2][:, qj % 2, :]
                        # d = p0/s0 - lam*p1/s1 — matching the reference's
                        # softmax normalization exactly (so LN_EPS compares
                        # against the same variance scale, and the exp bias
                        # e^-6 cancels)
                        r1 = small.tile([128, 1], F32, name="r1")
                        nc.vector.reciprocal(r1[:], p1[:, 128:129])
                        r0 = small.tile([128, 1], F32, name="r0")
                        nc.vector.reciprocal(r0[:], p0[:, 128:129])
                        negc = small.tile([128, 1], F32, name="negc")
                        nc.vector.tensor_tensor(
                            negc[:], neglam[:], r1[:], OP.mult)
                        dt_ = dtiles[h][qb * 4 + qj]
                        tmp = small.tile([128, 128], F32, name="tmp")
                        nc.vector.tensor_scalar(
                            tmp[:], p1[:, 0:128], negc[:], None, OP.mult)
                        nc.vector.scalar_tensor_tensor(
                            dt_[:], p0[:, 0:128], r0[:], tmp[:],
                            op0=OP.mult, op1=OP.add)
                        # (d * 1.0) * d with accumulated sum -> sum(d^2)
                        dsq = small.tile([128, 128], F16, name="dsq")
                        nc.vector.scalar_tensor_tensor(
                            dsq[:], dt_[:], 1.0, dt_[:],
                            op0=OP.mult, op1=OP.mult,
                            accum_out=sumsq[:, col:col + 1])
                    return epi

                def early(self, ps_log):
                    ks = 4 * self.qb
                    out = []
                    if ks == 0:
                        return out
                    out.append(self.mk_u1(0, ps_log))
                    for kc in range(1, ks):
                        out.append(self.mk_u1(kc, ps_log))
                        out.append(self.mk_u2(kc - 1))
                    # u2(ks-1) is deferred to the diagonal segment
                    return out

                def diag(self, ps_log):
                    ks = 4 * self.qb
                    out = []
                    for kc in range(ks, self.nck):
                        out.append(self.mk_u1(kc, ps_log))
                        if kc >= 1:
                            out.append(self.mk_u2(kc - 1))
                    out.append(self.mk_u2(self.nck - 1))
                    out += [self.mk_epi(qj) for qj in range(4)]
                    return out

            bgen = [[BGen(h, qb) for qb in range(QB)] for h in range(H_LOC)]

            def weave(quanta, bcl):
                n, m = len(bcl), max(1, len(quanta))
                bi = 0
                for i, q in enumerate(quanta):
                    q()
                    tgt = (i + 1) * n // m
                    while bi < tgt:
                        bcl[bi]()
                        bi += 1
                while bi < n:
                    bcl[bi]()
                    bi += 1

            # Wo prefetch ring: halves 0,1 stream in during tb3 (the sync
            # ring is idle then); 2,3 pipeline behind the out-proj
            wopool = ctx.enter_context(tc.tile_pool(name="wopool", bufs=2))
            wo_tiles = {}

            def wo_dma(do):
                def f():
                    wo_t = wopool.tile([128, KD, 512], F16, name="wo")
                    nc.sync.dma_start(
                        wo_t[:],
                        wo_d[:, do * 512:(do + 1) * 512].rearrange(
                            "(a p) c -> p a c", p=128))
                    wo_tiles[do] = wo_t
                return f

            # ---------------- phase A (woven with B) ----------------
            with ExitStack() as actx:
                wpool = actx.enter_context(tc.tile_pool(name="wpool", bufs=1))
                xchunk = actx.enter_context(
                    tc.tile_pool(name="xchunk", bufs=5))
                xtsp = actx.enter_context(tc.tile_pool(name="xtsp", bufs=2))
                ps_a = actx.enter_context(
                    tc.tile_pool(name="ps_a", bufs=2, space="PSUM"))
                ps_log_a = actx.enter_context(
                    tc.tile_pool(name="ps_log_a", bufs=2, space="PSUM"))

                # PE warmup: back-to-back transposes trip the HAM clock
                # gate to 8/8 before the first x chunk lands
                def keep_warm(n):
                    wps = ps_a.tile([128, 512], F32R, name="ps")
                    for i in range(n):
                        nc.tensor.transpose(
                            wps[:, (i % 4) * 128:(i % 4 + 1) * 128],
                            ident_r[:], ident_r[:])
                keep_warm(56)

                wq_t = wpool.tile([128, KD, 256], F16, name="wq")
                wk_t = wpool.tile([128, KD, 256], F16, name="wk")
                wv_t = wpool.tile([128, KD, 256], F16, name="wv")
                # halves interleaved so early dj chunks of all three arrive
                # first; gpsimd queue (scalar queue carries exps in phase B)
                for lo, hi in ((0, 8), (8, 16)):
                    for w_t, w_d in ((wq_t, wq_d), (wk_t, wk_d), (wv_t, wv_d)):
                        nc.gpsimd.dma_start(
                            w_t[:, lo:hi, :],
                            w_d[lo * 128:hi * 128, :].rearrange(
                                "(a p) c -> p a c", p=128))

                xts = [None, None]

                def xq(tb, tt):
                    ch = xchunk.tile([128, D_EMB], F32R, name="xch")
                    nc.sync.dma_start(
                        ch[:],
                        x_d[tb * 512 + tt * 128:tb * 512 + (tt + 1) * 128, :])
                    return ch

                # first t-block's chunks: first DMAs on the sync queue
                chunks = {(0, tt): xq(0, tt) for tt in range(4)}

                for tb in range(TB):
                    xts[tb % 2] = xtsp.tile([128, KD, 512], F16, name="xts")
                    xt = xts[tb % 2]

                    # ---- part 1: transposes + QK projections,
                    #      woven with (h, qb=tb-1) diagonal segments ----
                    quanta = []
                    for tt in range(4):
                        if tb == 0 and tt >= 1:
                            # dummy filler keeps the PE (and HAM) busy in
                            # the gaps between tb0's chunk arrivals
                            quanta.append(lambda: keep_warm(16))
                        for g in range(4):  # groups of 4 dj chunks
                            def tq(tt=tt, g=g, tb=tb, xt=xt):
                                ch = chunks[(tb, tt)]
                                pst = ps_a.tile([128, 512], F32R, name="ps")
                                for k in range(4):
                                    dj = g * 4 + k
                                    nc.tensor.transpose(
                                        pst[:, k * 128:(k + 1) * 128],
                                        ch[:, dj * 128:(dj + 1) * 128],
                                        ident_r[:])
                                nc.vector.tensor_copy(
                                    xt[:, g * 4:g * 4 + 4,
                                       tt * 128:(tt + 1) * 128],
                                    pst[:].rearrange("p (a c) -> p a c", a=4))
                            quanta.append(tq)
                        if tb + 1 < TB:
                            def pf(tb=tb, tt=tt):
                                chunks[(tb + 1, tt)] = xq(tb + 1, tt)
                            quanta.append(pf)

                    for h in range(H_LOC):
                        for w_t, dstq in ((wq_t, QT), (wk_t, KT)):
                            def qk(w_t=w_t, dstq=dstq, h=h, tb=tb, xt=xt):
                                psq = ps_a.tile([128, 512], F32, name="ps")
                                for dj in range(KD):
                                    nc.tensor.matmul(
                                        psq[:],
                                        w_t[:, dj, h * 128:(h + 1) * 128],
                                        xt[:, dj, :],
                                        start=(dj == 0), stop=(dj == KD - 1))
                                nc.vector.tensor_copy(
                                    dstq[h][:, tb * 512:(tb + 1) * 512],
                                    psq[:])
                            quanta.append(qk)

                    # PV psum holds exactly ONE (h, qb) accumulator set, so
                    # the chains are strictly sequenced: ..., D(h0,qb),
                    # E(h1,qb), D(h1,qb), E(h0,qb+1), ... staggered across
                    # the part1/part2 weaves.
                    bcl = []
                    if tb >= 1:
                        bcl = (bgen[0][tb - 1].diag(ps_log_a)
                               + bgen[1][tb - 1].early(ps_log_a))
                    if tb == TB - 1:
                        bcl.insert(len(bcl) // 2, wo_dma(0))
                    weave(quanta, bcl)

                    # ---- part 2: V projections, woven with the
                    #      (h, qb=tb) early segments ----
                    quanta = []
                    for tt in range(4):
                        def vq(tt=tt, tb=tb, xt=xt):
                            psv = ps_a.tile([128, 256], F32, name="ps")
                            for dj in range(KD):
                                nc.tensor.matmul(
                                    psv[:],
                                    xt[:, dj, tt * 128:(tt + 1) * 128],
                                    wv_t[:, dj, :],
                                    start=(dj == 0), stop=(dj == KD - 1))
                            vt = V[tb * 4 + tt]
                            nc.scalar.copy(
                                vt[:, :, 0:128],
                                psv[:].rearrange("p (h c) -> p h c", h=2))
                            # center V rows over dv: LN(d) is exactly
                            # invariant (softmax rows sum to 1), removes the
                            # near-constant row component that otherwise
                            # amplifies fp16 rounding ~50x through 1/sigma of
                            # rows where the two streams nearly cancel, and
                            # makes mean_dv(d) exactly 0 so B.5 skips mu
                            vsum = small.tile([128, 2], F32, name="vsum")
                            nc.vector.tensor_reduce(
                                out=vsum[:], in_=vt[:, :, 0:128],
                                axis=mybir.AxisListType.X, op=OP.add)
                            nmean = small.tile([128, 2], F32, name="nmean")
                            nc.vector.tensor_scalar(
                                nmean[:], vsum[:], -1.0 / 128.0, None,
                                OP.mult)
                            for hh in range(H_LOC):
                                nc.vector.tensor_scalar(
                                    vt[:, hh, 0:128], vt[:, hh, 0:128],
                                    nmean[:, hh:hh + 1], None, OP.add)
                        quanta.append(vq)

                    bcl = []
                    if tb >= 1:
                        bcl += bgen[1][tb - 1].diag(ps_log_a)
                    bcl += bgen[0][tb].early(ps_log_a)
                    if tb == TB - 1:
                        bcl.insert(len(bcl) // 2, wo_dma(1))
                    weave(quanta, bcl)

            # rstd for one head's 16 columns (one Ln + one Exp):
            # rstd' = (1-li)/sqrt(var+eps) = exp(-.5*ln(var+eps)+lnc);
            # var = sumsq/128 (mean is exactly 0 by V-centering)
            def rstd_head(h):
                def f():
                    sl = slice(h * 16, h * 16 + 16)
                    nc.vector.tensor_scalar(
                        varp[:, sl], sumsq[:, sl], 1.0 / 128.0, None, OP.mult)
                    nc.scalar.activation(lnv[:, sl], varp[:, sl], AF.Ln,
                                         bias=eps_t[:])
                    nc.scalar.activation(rstd[:, sl], lnv[:, sl], AF.Exp,
                                         scale=-0.5, bias=lnc_t[:])
                return f

            # B.5: MT[h] = (d * rstd)^T via a REAL matmul with a
            # diag(rstd) rhs (the transpose datapath ignores its rhs)
            def gen_b5_quanta(h, ps_c):
                quanta = []
                for qb in range(QB):
                    def b5q(h=h, qb=qb):
                        mt_ps = ps_c.tile([128, 512], F32, name="psc")
                        for qj in range(4):
                            i = qb * 4 + qj
                            col = h * 16 + i
                            diag = small.tile([128, 128], F16, name="diag")
                            nc.vector.tensor_scalar(
                                diag[:], ident_h[:], rstd[:, col:col + 1],
                                None, OP.mult)
                            nc.tensor.matmul(
                                mt_ps[:, qj * 128:(qj + 1) * 128],
                                dtiles[h][i][:], diag[:],
                                start=True, stop=True)
                        nc.vector.tensor_copy(
                            MT[h][:, qb * 512:(qb + 1) * 512], mt_ps[:])
                    quanta.append(b5q)
                return quanta

            def gen_c_quanta(h, dos, ps_c):
                quanta = []
                for do in dos:
                    def cq(h=h, do=do):
                        wo_t = wo_tiles[do]
                        ys = ps_c.tile([128, 512], F32, name="psc")
                        mt_r = MT[h][:].rearrange("p (m g) -> p g m", g=16)
                        for j in range(KD):
                            nc.tensor.matmul(
                                ys[:], mt_r[:, j, :], wo_t[:, j, :],
                                start=(j == 0), stop=(j == KD - 1))
                        y_sb = ypool.tile([128, 512], F32, name="ysb")
                        nc.vector.tensor_copy(y_sb[:], ys[:])
                        nc.sync.dma_start(
                            y_d[h * 128:(h + 1) * 128,
                                do * 512:(do + 1) * 512],
                            y_sb[:])
                    quanta.append(cq)
                return quanta

            # ---- tail A: h0's qb=3 diagonal (short, lookahead via 4-buf
            #      logits ring) ----
            with ExitStack() as t0ctx:
                ps_log_t0 = t0ctx.enter_context(
                    tc.tile_pool(name="ps_log_t0", bufs=4, space="PSUM"))
                for f in bgen[0][3].diag(ps_log_t0):
                    f()
                # keep the PE busy through h0's epilogue lull
                wps = ps_log_t0.tile([128, 512], F32R, name="pslog")
                for i in range(16):
                    nc.tensor.transpose(
                        wps[:, (i % 4) * 128:(i % 4 + 1) * 128],
                        ident_r[:], ident_r[:])

            # ---- tail B: h1's full qb=3 chain, woven with B.5(h0) and
            #      out-proj(h0) as PE filler ----
            with ExitStack() as t1ctx:
                ps_log_t1 = t1ctx.enter_context(
                    tc.tile_pool(name="ps_log_t1", bufs=3, space="PSUM"))
                ps_c1 = t1ctx.enter_context(
                    tc.tile_pool(name="ps_c1", bufs=1, space="PSUM"))
                steps = (bgen[1][3].early(ps_log_t1)
                         + bgen[1][3].diag(ps_log_t1))
                c0 = gen_c_quanta(0, [0, 1, 2, 3], ps_c1)
                bcl = [rstd_head(0)]
                bcl += gen_b5_quanta(0, ps_c1)
                bcl += [c0[0], c0[1], wo_dma(2), c0[2], wo_dma(3), c0[3]]
                weave(steps, bcl)

            # ---- tail C: B.5(h1) + out-proj(h1); wo halves 2,3 are
            #      still resident, 0,1 re-stream behind the matmuls ----
            with ExitStack() as cctx:
                ps_cf = cctx.enter_context(
                    tc.tile_pool(name="ps_cf", bufs=2, space="PSUM"))
                rstd_head(1)()
                for q in gen_b5_quanta(1, ps_cf):
                    q()
                c1 = gen_c_quanta(1, [2, 3, 0, 1], ps_cf)
                c1[0]()
                c1[1]()
                wo_dma(0)()
                c1[2]()
                wo_dma(1)()
                c1[3]()

    nc.compile()
    return nc


_NC_CACHE = None


def make_in_maps(x, Wq, Wk, Wv, Wo, lambda_q1, lambda_k1, lambda_q2,
                 lambda_k2):
    x2 = np.ascontiguousarray(
        np.asarray(x, np.float32).reshape(T, D_EMB))
    Wq16 = np.asarray(Wq, np.float32).astype(np.float16)
    Wk16 = np.asarray(Wk, np.float32).astype(np.float16)
    Wv16 = np.asarray(Wv, np.float32).astype(np.float16)
    Wo16 = np.ascontiguousarray(np.asarray(Wo, np.float32).astype(np.float16))

    lam = (math.exp(float(np.dot(np.asarray(lambda_q1, np.float64),
                                 np.asarray(lambda_k1, np.float64))))
           - math.exp(float(np.dot(np.asarray(lambda_q2, np.float64),
                                   np.asarray(lambda_k2, np.float64))))
           + LAMBDA_INIT)
    neglam = np.full((128, 1), -lam, dtype=np.float32)

    in_maps = []
    for m in range(N_CORES):
        sl = slice(256 * m, 256 * (m + 1))
        in_maps.append({
            "x": x2,
            "wq": np.ascontiguousarray(Wq16[:, sl]),
            "wk": np.ascontiguousarray(Wk16[:, sl]),
            "wv": np.ascontiguousarray(Wv16[:, sl]),
            "wo": Wo16,
            "neglam": neglam,
        })
    return in_maps


def kernel(x, Wq, Wk, Wv, Wo, lambda_q1, lambda_k1, lambda_q2, lambda_k2):
    global _NC_CACHE
    if _NC_CACHE is None:
        _NC_CACHE = _build_program()
    nc = _NC_CACHE

    B = np.asarray(x).shape[0]
    in_maps = make_in_maps(x, Wq, Wk, Wv, Wo, lambda_q1, lambda_k1,
                           lambda_q2, lambda_k2)
    res = run_bass_kernel_spmd(nc, in_maps, list(range(N_CORES)))
    y = np.concatenate([res.results[m]["y"] for m in range(N_CORES)], axis=0)
    return y.reshape(B, T, D_EMB)


# revision 31
# speedup vs baseline: 1.1300x; 1.0472x over previous
"""DiffAttn (Differential Transformer attention) on 8 trn2 NeuronCores.

Sharding: tensor-parallel over heads. 16 heads / 8 cores = 2 heads per core.
Wq/Wk/Wv column-sharded (256 cols/core), Wo + x replicated. The reference's
"reshape without transposing heads back" maps output row r = h*128 + (t//16)
entirely to head h, so each core produces rows [256*m, 256*(m+1)) of the
final (2048, 2048) output with NO collectives.

v4 design notes (perf, from v2/v3 NTFF traces):
- The HAM clock gate needs ~3.4us of CONTINUOUS PE busy to reach the
  2.4GHz state and drops back after any ~3.4us lull. v3 lost ~48us to
  half-clock: the serial qb=3 tail (33 kc-step chains with a ~2.2us
  latency chain per step) never re-promoted.
- v4 splits each (h, qb) attention chain at its diagonal: the early
  segment (kc < 4qb) only needs K/V blocks < qb, so it weaves into
  t-block qb's OWN quanta (right after that block's QT/KT are formed);
  the 4 diagonal steps weave into t-block qb+1. The "tail" shrinks to
  the two heads' qb=3 diagonals (interleaved, lookahead via zip),
  epilogues, LN-fold, and the out-projection.
- fp16 everywhere post-transpose (host-cast W, x^T cast in the
  PSUM->SBUF copy): fp16 LDWEIGHTS gets FWL and hides behind streams;
  fp32 LDW (150ns, unhidable) cost v2 77us.
- Start: ~56 warmup transposes trip HAM immediately; x arrives in
  [128,2048] chunks with per-chunk transpose quanta + dummy-transpose
  filler covering the chunk-DMA gaps of the first t-block.
- LN: mean over dv is exactly 0 by V-centering (softmax rows sum to 1)
  so mu is dropped; rstd folds into the d^T transpose as a REAL matmul
  with a diag(rstd) rhs (transpose-mode ignores its rhs).
- Wo fully resident (bufs=4) so both heads' out-proj share each wo_t
  with zero re-DMA; weight/Wo/y DMAs keep off the scalar queue (ACT
  runs the exp chain).
"""
import math
from contextlib import ExitStack

import numpy as np

import concourse.bass as bass
import concourse.mybir as mybir
import concourse.tile as tile
from concourse import bacc
from concourse.bass_utils import run_bass_kernel_spmd
from concourse.masks import make_identity

F32 = mybir.dt.float32
F32R = mybir.dt.float32r
F16 = mybir.dt.float16
AF = mybir.ActivationFunctionType
OP = mybir.AluOpType

T = 2048
D_EMB = 2048
HD = 64           # head dim per stream
N_CORES = 8
H_LOC = 2         # heads per core
SCALE = HD ** -0.5
LAMBDA_INIT = 0.8 - 0.6 * math.exp(-0.3 * 12)
LN_EPS = 1e-5

TB = 4            # t-blocks of 512 in phase A
QB = 4            # q-blocks of 512 in phase B
KD = 16           # contraction chunks of 128 over D_EMB


def _build_program():
    nc = bacc.Bacc("TRN2", target_bir_lowering=False, debug=False)

    x_d = nc.dram_tensor("x", [T, D_EMB], F32R, kind="ExternalInput").ap()
    wq_d = nc.dram_tensor("wq", [D_EMB, 256], F16, kind="ExternalInput").ap()
    wk_d = nc.dram_tensor("wk", [D_EMB, 256], F16, kind="ExternalInput").ap()
    wv_d = nc.dram_tensor("wv", [D_EMB, 256], F16, kind="ExternalInput").ap()
    wo_d = nc.dram_tensor("wo", [D_EMB, D_EMB], F16, kind="ExternalInput").ap()
    neglam_d = nc.dram_tensor("neglam", [128, 1], F32, kind="ExternalInput").ap()
    y_d = nc.dram_tensor("y", [256, D_EMB], F32, kind="ExternalOutput").ap()

    with tile.TileContext(nc) as tc:
        with ExitStack() as ctx:
            const = ctx.enter_context(tc.tile_pool(name="const", bufs=1))
            qkv = ctx.enter_context(tc.tile_pool(name="qkv", bufs=1))
            dst = ctx.enter_context(tc.tile_pool(name="dst", bufs=1))
            small = ctx.enter_context(tc.tile_pool(name="small", bufs=4))
            ypool = ctx.enter_context(tc.tile_pool(name="ypool", bufs=2))
            etp = ctx.enter_context(tc.tile_pool(name="etp", bufs=6))
            ps_pv0 = ctx.enter_context(
                tc.tile_pool(name="ps_pv0", bufs=2, space="PSUM"))
            ps_pv1 = ctx.enter_context(
                tc.tile_pool(name="ps_pv1", bufs=2, space="PSUM"))

            # ---------------- constants ----------------
            ident = const.tile([128, 128], F32)
            make_identity(nc, ident[:])
            ident_r = const.tile([128, 128], F32R)
            nc.scalar.copy(ident_r[:], ident[:])
            ident_h = const.tile([128, 128], F16)
            nc.scalar.copy(ident_h[:], ident[:])
            neglam = const.tile([128, 1], F32)
            nc.scalar.dma_start(neglam[:], neglam_d)
            eps_t = const.tile([128, 1], F32)
            nc.gpsimd.memset(eps_t[:], LN_EPS)
            lnc_t = const.tile([128, 1], F32)
            nc.gpsimd.memset(lnc_t[:], math.log(1.0 - LAMBDA_INIT))
            ebias_t = const.tile([128, 1], F32)
            nc.gpsimd.memset(ebias_t[:], -6.0)
            # tri[k, q] = 1 where k <= q else 0 (causal keep-mask, fp16)
            tri = const.tile([128, 128], F16)
            nc.gpsimd.memset(tri[:], 1.0)
            nc.gpsimd.affine_select(
                out=tri[:], in_=tri[:], compare_op=OP.is_ge, fill=0.0,
                base=0, pattern=[[1, 128]], channel_multiplier=-1)

            # weight DMAs as early as possible, split across the gpsimd
            # and vector rings (each direct2d issue occupies its engine
            # ~1-2us; one ring would delay the high halves past the first
            # QK projections)
            wpool = ctx.enter_context(tc.tile_pool(name="wpool", bufs=1))
            wq_t = wpool.tile([128, KD, 256], F16, name="wq")
            wk_t = wpool.tile([128, KD, 256], F16, name="wk")
            wv_t = wpool.tile([128, KD, 256], F16, name="wv")
            for i, (w_t, w_d, lo, hi) in enumerate((
                    (wq_t, wq_d, 0, 8), (wk_t, wk_d, 0, 8),
                    (wv_t, wv_d, 0, 8), (wq_t, wq_d, 8, 16),
                    (wk_t, wk_d, 8, 16), (wv_t, wv_d, 8, 16))):
                eng = nc.gpsimd if i % 2 == 0 else nc.scalar
                eng.dma_start(
                    w_t[:, lo:hi, :],
                    w_d[lo * 128:hi * 128, :].rearrange(
                        "(a p) c -> p a c", p=128))

            # ---------------- persistent tensors ----------------
            QT = [qkv.tile([128, T], F16, name=f"qt{h}") for h in range(H_LOC)]
            KT = [qkv.tile([128, T], F16, name=f"kt{h}") for h in range(H_LOC)]
            # V[t]: [k(128), head(2), 132]; cols 0:128 = V data, col 128 = 1.0
            V = [qkv.tile([128, 2, 132], F16, name=f"v{t}") for t in range(16)]
            MT = [qkv.tile([128, T], F16, name=f"mt{h}") for h in range(H_LOC)]
            for t in range(16):
                nc.gpsimd.memset(V[t][:, :, 128:129], 1.0)

            # d staging + LN statistics (col = h*16 + qb*4 + qj)
            dtiles = [[dst.tile([128, 128], F16, name=f"d{h}_{i}")
                       for i in range(16)] for h in range(H_LOC)]
            sumsq = dst.tile([128, 32], F32, name="sumsq")
            varp = dst.tile([128, 32], F32, name="varp")
            lnv = dst.tile([128, 32], F32, name="lnv")
            rstd = dst.tile([128, 32], F32, name="rstd")

            # ---------------- phase B chain generator ----------------
            class BGen:
                """One (h, qb) attention chain, split at the diagonal.

                early(): closures for kc in [0, 4qb) at lookahead-1
                         (only needs K/V t-blocks < qb).
                diag():  closures for kc in [4qb, 4qb+4), the trailing
                         PV of the early segment, and the 4 per-qj
                         epilogues.
                """

                def __init__(self, h, qb):
                    self.h, self.qb = h, qb
                    self.nck = 4 * qb + 4
                    self.st = {}

                def mk_u1(self, kc, ps_log):
                    h, qb = self.h, self.qb

                    def u1():
                        if kc == 0:
                            self.st["pv"] = [
                                [ps_pv0.tile([128, 2, 136], F32, name="pv0")
                                 for _ in range(2)],
                                [ps_pv1.tile([128, 2, 136], F32, name="pv1")
                                 for _ in range(2)],
                            ]
                        j = kc - 4 * qb
                        qs = 128 * j if j > 0 else 0
                        ets = []
                        for s in (0, 1):
                            stp = ps_log.tile([128, 512], F32, name="pslog")
                            nc.tensor.matmul(
                                stp[:, qs:512],
                                KT[h][s * 64:(s + 1) * 64,
                                      kc * 128:(kc + 1) * 128],
                                QT[h][s * 64:(s + 1) * 64,
                                      qb * 512 + qs:(qb + 1) * 512],
                                start=True, stop=True)
                            # bias -6 keeps exp and the E*V products in fp16
                            # range; the softmax ratio and LayerNorm are
                            # invariant to the uniform e^-6 factor
                            et = etp.tile([128, 512], F16, name="et")
                            nc.scalar.activation(et[:, qs:512], stp[:, qs:512],
                                                 AF.Exp, scale=SCALE,
                                                 bias=ebias_t[:])
                            if j >= 0:
                                nc.gpsimd.tensor_tensor(
                                    et[:, qs:qs + 128], et[:, qs:qs + 128],
                                    tri[:], OP.mult)
                            ets.append(et)
                        self.st[kc] = ets
                    return u1

                def mk_u2(self, kc):
                    h, qb = self.h, self.qb

                    def u2():
                        j = kc - 4 * qb
                        ets = self.st.pop(kc)
                        pv = self.st["pv"]
                        for s in (0, 1):
                            for qj in range(4):
                                if j > qj:
                                    continue
                                # start=True zeroes the WHOLE 2KB psum zero
                                # region, so only the first matmul into each
                                # bank starts; the odd-qj group's first write
                                # lands on pending-zero bytes and overwrites.
                                nc.tensor.matmul(
                                    pv[s][qj // 2][:, qj % 2, 0:129],
                                    ets[s][:, qj * 128:(qj + 1) * 128],
                                    V[kc][:, h, 0:129],
                                    start=(kc == 0 and qj % 2 == 0),
                                    stop=(kc == 4 * qb + qj),
                                    skip_group_check=True)
                    return u2

                def mk_epi(self, qj):
                    h, qb = self.h, self.qb

                    def epi():
                        pv = self.st["pv"] if qj < 3 else self.st.pop("pv")
                        col = h * 16 + qb * 4 + qj
                        p0 = pv[0][qj // 2][:, qj % 2, :]
                        p1 = pv[1][qj // 2][:, qj % 2, :]
                        # d = p0/s0 - lam*p1/s1 — matching the reference's
                        # softmax normalization exactly (so LN_EPS compares
                        # against the same variance scale, and the exp bias
                        # e^-6 cancels)
                        r1 = small.tile([128, 1], F32, name="r1")
                        nc.vector.reciprocal(r1[:], p1[:, 128:129])
                        r0 = small.tile([128, 1], F32, name="r0")
                        nc.vector.reciprocal(r0[:], p0[:, 128:129])
                        negc = small.tile([128, 1], F32, name="negc")
                        nc.vector.tensor_tensor(
                            negc[:], neglam[:], r1[:], OP.mult)
                        dt_ = dtiles[h][qb * 4 + qj]
                        tmp = small.tile([128, 128], F32, name="tmp")
                        nc.vector.tensor_scalar(
                            tmp[:], p1[:, 0:128], negc[:], None, OP.mult)
                        nc.vector.scalar_tensor_tensor(
                            dt_[:], p0[:, 0:128], r0[:], tmp[:],
                            op0=OP.mult, op1=OP.add)
                        # (d * 1.0) * d with accumulated sum -> sum(d^2)
                        dsq = small.tile([128, 128], F16, name="dsq")
                        nc.vector.scalar_tensor_tensor(
                            dsq[:], dt_[:], 1.0, dt_[:],
                            op0=OP.mult, op1=OP.mult,
                            accum_out=sumsq[:, col:col + 1])
                    return epi

                def early(self, ps_log):
                    ks = 4 * self.qb
                    out = []
                    if ks == 0:
                        return out
                    out.append(self.mk_u1(0, ps_log))
                    for kc in range(1, ks):
                        out.append(self.mk_u1(kc, ps_log))
                        out.append(self.mk_u2(kc - 1))
                    # u2(ks-1) is deferred to the diagonal segment
                    return out

                def diag(self, ps_log):
                    ks = 4 * self.qb
                    out = []
                    for kc in range(ks, self.nck):
                        out.append(self.mk_u1(kc, ps_log))
                        if kc >= 1:
                            out.append(self.mk_u2(kc - 1))
                    out.append(self.mk_u2(self.nck - 1))
                    out += [self.mk_epi(qj) for qj in range(4)]
                    return out

            bgen = [[BGen(h, qb) for qb in range(QB)] for h in range(H_LOC)]

            def weave(quanta, bcl):
                n, m = len(bcl), max(1, len(quanta))
                bi = 0
                for i, q in enumerate(quanta):
                    q()
                    tgt = (i + 1) * n // m
                    while bi < tgt:
                        bcl[bi]()
                        bi += 1
                while bi < n:
                    bcl[bi]()
                    bi += 1

            # Wo prefetch ring: halves 0,1 stream in during tb3 (the sync
            # ring is idle then); 2,3 pipeline behind the out-proj
            wopool = ctx.enter_context(tc.tile_pool(name="wopool", bufs=3))
            wo_tiles = {}

            def wo_dma(do):
                def f():
                    wo_t = wopool.tile([128, KD, 512], F16, name="wo")
                    nc.sync.dma_start(
                        wo_t[:],
                        wo_d[:, do * 512:(do + 1) * 512].rearrange(
                            "(a p) c -> p a c", p=128))
                    wo_tiles[do] = wo_t
                return f

            # ---------------- phase A (woven with B) ----------------
            with ExitStack() as actx:
                xchunk = actx.enter_context(
                    tc.tile_pool(name="xchunk", bufs=4))
                xtsp = actx.enter_context(tc.tile_pool(name="xtsp", bufs=2))
                ps_a = actx.enter_context(
                    tc.tile_pool(name="ps_a", bufs=2, space="PSUM"))
                ps_log_a = actx.enter_context(
                    tc.tile_pool(name="ps_log_a", bufs=2, space="PSUM"))

                # PE warmup: back-to-back transposes trip the HAM clock
                # gate to 8/8 before the first x chunk lands
                def keep_warm(n):
                    wps = ps_a.tile([128, 512], F32R, name="ps")
                    for i in range(n):
                        nc.tensor.transpose(
                            wps[:, (i % 4) * 128:(i % 4 + 1) * 128],
                            ident_r[:], ident_r[:])
                keep_warm(56)

                xts = [None, None]

                def xq(tb, tt):
                    ch = xchunk.tile([128, D_EMB], F32R, name="xch")
                    nc.sync.dma_start(
                        ch[:],
                        x_d[tb * 512 + tt * 128:tb * 512 + (tt + 1) * 128, :])
                    return ch

                # first t-block's chunks: first DMAs on the sync queue
                chunks = {(0, tt): xq(0, tt) for tt in range(4)}

                for tb in range(TB):
                    xts[tb % 2] = xtsp.tile([128, KD, 512], F16, name="xts")
                    xt = xts[tb % 2]

                    # ---- part 1: transposes + QK projections,
                    #      woven with (h, qb=tb-1) diagonal segments ----
                    quanta = []
                    for tt in range(4):
                        if tb == 0 and tt >= 1:
                            # dummy filler keeps the PE (and HAM) busy in
                            # the gaps between tb0's chunk arrivals
                            quanta.append(lambda: keep_warm(16))
                        for g in range(4):  # groups of 4 dj chunks
                            def tq(tt=tt, g=g, tb=tb, xt=xt):
                                ch = chunks[(tb, tt)]
                                pst = ps_a.tile([128, 512], F32R, name="ps")
                                for k in range(4):
                                    dj = g * 4 + k
                                    nc.tensor.transpose(
                                        pst[:, k * 128:(k + 1) * 128],
                                        ch[:, dj * 128:(dj + 1) * 128],
                                        ident_r[:])
                                nc.vector.tensor_copy(
                                    xt[:, g * 4:g * 4 + 4,
                                       tt * 128:(tt + 1) * 128],
                                    pst[:].rearrange("p (a c) -> p a c", a=4))
                            quanta.append(tq)
                        if tb + 1 < TB:
                            def pf(tb=tb, tt=tt):
                                chunks[(tb + 1, tt)] = xq(tb + 1, tt)
                            quanta.append(pf)

                    for h in range(H_LOC):
                        for w_t, dstq in ((wq_t, QT), (wk_t, KT)):
                            def qk(w_t=w_t, dstq=dstq, h=h, tb=tb, xt=xt):
                                psq = ps_a.tile([128, 512], F32, name="ps")
                                for dj in range(KD):
                                    nc.tensor.matmul(
                                        psq[:],
                                        w_t[:, dj, h * 128:(h + 1) * 128],
                                        xt[:, dj, :],
                                        start=(dj == 0), stop=(dj == KD - 1))
                                nc.vector.tensor_copy(
                                    dstq[h][:, tb * 512:(tb + 1) * 512],
                                    psq[:])
                            quanta.append(qk)

                    # PV psum holds exactly ONE (h, qb) accumulator set, so
                    # the chains are strictly sequenced: ..., D(h0,qb),
                    # E(h1,qb), D(h1,qb), E(h0,qb+1), ... staggered across
                    # the part1/part2 weaves.
                    bcl = []
                    if tb >= 1:
                        bcl = (bgen[0][tb - 1].diag(ps_log_a)
                               + bgen[1][tb - 1].early(ps_log_a))
                    if tb == TB - 1:
                        bcl.insert(len(bcl) // 2, wo_dma(0))
                    weave(quanta, bcl)

                    # ---- part 2: V projections, woven with the
                    #      (h, qb=tb) early segments ----
                    quanta = []
                    for tt in range(4):
                        def vq(tt=tt, tb=tb, xt=xt):
                            psv = ps_a.tile([128, 256], F32, name="ps")
                            for dj in range(KD):
                                nc.tensor.matmul(
                                    psv[:],
                                    xt[:, dj, tt * 128:(tt + 1) * 128],
                                    wv_t[:, dj, :],
                                    start=(dj == 0), stop=(dj == KD - 1))
                            vt = V[tb * 4 + tt]
                            nc.scalar.copy(
                                vt[:, :, 0:128],
                                psv[:].rearrange("p (h c) -> p h c", h=2))
                            # center V rows over dv: LN(d) is exactly
                            # invariant (softmax rows sum to 1), removes the
                            # near-constant row component that otherwise
                            # amplifies fp16 rounding ~50x through 1/sigma of
                            # rows where the two streams nearly cancel, and
                            # makes mean_dv(d) exactly 0 so B.5 skips mu
                            vsum = small.tile([128, 2], F32, name="vsum")
                            nc.vector.tensor_reduce(
                                out=vsum[:], in_=vt[:, :, 0:128],
                                axis=mybir.AxisListType.X, op=OP.add)
                            nmean = small.tile([128, 2], F32, name="nmean")
                            nc.vector.tensor_scalar(
                                nmean[:], vsum[:], -1.0 / 128.0, None,
                                OP.mult)
                            for hh in range(H_LOC):
                                nc.vector.tensor_scalar(
                                    vt[:, hh, 0:128], vt[:, hh, 0:128],
                                    nmean[:, hh:hh + 1], None, OP.add)
                        quanta.append(vq)

                    bcl = []
                    if tb >= 1:
                        bcl += bgen[1][tb - 1].diag(ps_log_a)
                    bcl += bgen[0][tb].early(ps_log_a)
                    if tb == TB - 1:
                        bcl.insert(len(bcl) // 2, wo_dma(1))
                    weave(quanta, bcl)

                # ---- tail A (still inside the phase-A pools): h0's qb=3
                #      diagonal, then keep-warm through its epilogue ----
                for f in bgen[0][3].diag(ps_log_a):
                    f()
                keep_warm(16)

            # rstd for one head's 16 columns (one Ln + one Exp):
            # rstd' = (1-li)/sqrt(var+eps) = exp(-.5*ln(var+eps)+lnc);
            # var = sumsq/128 (mean is exactly 0 by V-centering)
            def rstd_head(h):
                def f():
                    sl = slice(h * 16, h * 16 + 16)
                    nc.vector.tensor_scalar(
                        varp[:, sl], sumsq[:, sl], 1.0 / 128.0, None, OP.mult)
                    nc.scalar.activation(lnv[:, sl], varp[:, sl], AF.Ln,
                                         bias=eps_t[:])
                    nc.scalar.activation(rstd[:, sl], lnv[:, sl], AF.Exp,
                                         scale=-0.5, bias=lnc_t[:])
                return f

            # B.5: MT[h] = (d * rstd)^T via a REAL matmul with a
            # diag(rstd) rhs (the transpose datapath ignores its rhs)
            def gen_b5_quanta(h, ps_c):
                quanta = []
                for qb in range(QB):
                    def b5q(h=h, qb=qb):
                        mt_ps = ps_c.tile([128, 512], F32, name="psc")
                        for qj in range(4):
                            i = qb * 4 + qj
                            col = h * 16 + i
                            diag = small.tile([128, 128], F16, name="diag")
                            nc.vector.tensor_scalar(
                                diag[:], ident_h[:], rstd[:, col:col + 1],
                                None, OP.mult)
                            nc.tensor.matmul(
                                mt_ps[:, qj * 128:(qj + 1) * 128],
                                dtiles[h][i][:], diag[:],
                                start=True, stop=True)
                        nc.vector.tensor_copy(
                            MT[h][:, qb * 512:(qb + 1) * 512], mt_ps[:])
                    quanta.append(b5q)
                return quanta

            def gen_c_quanta(h, dos, ps_c):
                quanta = []
                for do in dos:
                    def cq(h=h, do=do):
                        wo_t = wo_tiles[do]
                        ys = ps_c.tile([128, 512], F32, name="psc")
                        mt_r = MT[h][:].rearrange("p (m g) -> p g m", g=16)
                        for j in range(KD):
                            nc.tensor.matmul(
                                ys[:], mt_r[:, j, :], wo_t[:, j, :],
                                start=(j == 0), stop=(j == KD - 1))
                        y_sb = ypool.tile([128, 512], F32, name="ysb")
                        nc.vector.tensor_copy(y_sb[:], ys[:])
                        nc.sync.dma_start(
                            y_d[h * 128:(h + 1) * 128,
                                do * 512:(do + 1) * 512],
                            y_sb[:])
                    quanta.append(cq)
                return quanta

            # ---- tail B: h1's full qb=3 chain, woven with B.5(h0) and
            #      out-proj(h0) as PE filler ----
            with ExitStack() as t1ctx:
                ps_log_t1 = t1ctx.enter_context(
                    tc.tile_pool(name="ps_log_t1", bufs=3, space="PSUM"))
                ps_c1 = t1ctx.enter_context(
                    tc.tile_pool(name="ps_c1", bufs=1, space="PSUM"))
                steps = (bgen[1][3].early(ps_log_t1)
                         + bgen[1][3].diag(ps_log_t1))
                c0 = gen_c_quanta(0, [0, 1, 2, 3], ps_c1)
                bcl = [rstd_head(0)]
                bcl += gen_b5_quanta(0, ps_c1)
                bcl += [c0[0], c0[1], wo_dma(2), c0[2], wo_dma(3), c0[3]]
                weave(steps, bcl)
                # keep the PE busy through h1's epilogue + rstd lull
                wps = ps_log_t1.tile([128, 512], F32R, name="pslog")
                for i in range(24):
                    nc.tensor.transpose(
                        wps[:, (i % 4) * 128:(i % 4 + 1) * 128],
                        ident_r[:], ident_r[:])

            # ---- tail C: B.5(h1) + out-proj(h1); wo halves 2,3 are
            #      still resident, 0,1 re-stream behind the matmuls ----
            with ExitStack() as cctx:
                ps_cf = cctx.enter_context(
                    tc.tile_pool(name="ps_cf", bufs=2, space="PSUM"))
                rstd_head(1)()
                for q in gen_b5_quanta(1, ps_cf):
                    q()
                # wopool slots after c0: A=wo3, B=wo1, C=wo2; only wo0
                # needs a re-stream, hidden behind do=3 and do=2
                c1 = gen_c_quanta(1, [3, 1, 2, 0], ps_cf)
                c1[0]()
                c1[1]()
                wo_dma(0)()
                c1[2]()
                c1[3]()

    nc.compile()
    return nc


_NC_CACHE = None


def make_in_maps(x, Wq, Wk, Wv, Wo, lambda_q1, lambda_k1, lambda_q2,
                 lambda_k2):
    x2 = np.ascontiguousarray(
        np.asarray(x, np.float32).reshape(T, D_EMB))
    Wq16 = np.asarray(Wq, np.float32).astype(np.float16)
    Wk16 = np.asarray(Wk, np.float32).astype(np.float16)
    Wv16 = np.asarray(Wv, np.float32).astype(np.float16)
    Wo16 = np.ascontiguousarray(np.asarray(Wo, np.float32).astype(np.float16))

    lam = (math.exp(float(np.dot(np.asarray(lambda_q1, np.float64),
                                 np.asarray(lambda_k1, np.float64))))
           - math.exp(float(np.dot(np.asarray(lambda_q2, np.float64),
                                   np.asarray(lambda_k2, np.float64))))
           + LAMBDA_INIT)
    neglam = np.full((128, 1), -lam, dtype=np.float32)

    in_maps = []
    for m in range(N_CORES):
        sl = slice(256 * m, 256 * (m + 1))
        in_maps.append({
            "x": x2,
            "wq": np.ascontiguousarray(Wq16[:, sl]),
            "wk": np.ascontiguousarray(Wk16[:, sl]),
            "wv": np.ascontiguousarray(Wv16[:, sl]),
            "wo": Wo16,
            "neglam": neglam,
        })
    return in_maps


def kernel(x, Wq, Wk, Wv, Wo, lambda_q1, lambda_k1, lambda_q2, lambda_k2):
    global _NC_CACHE
    if _NC_CACHE is None:
        _NC_CACHE = _build_program()
    nc = _NC_CACHE

    B = np.asarray(x).shape[0]
    in_maps = make_in_maps(x, Wq, Wk, Wv, Wo, lambda_q1, lambda_k1,
                           lambda_q2, lambda_k2)
    res = run_bass_kernel_spmd(nc, in_maps, list(range(N_CORES)))
    y = np.concatenate([res.results[m]["y"] for m in range(N_CORES)], axis=0)
    return y.reshape(B, T, D_EMB)
